# revision 30
# baseline (speedup 1.0000x reference)
"""SAM-block (windowed attention + MLP) + FRM fusion on 8 TRN2 NeuronCores.

v2: LN affines folded into weights (host), multiplicative rel-pos bias
exp(S)*exp(gh)*exp(gw), softmax denominator via ones-column in V,
scalar_tensor_tensor fusions, rgb stats in phase 0, sw gate rows folded
into the P4 channel matmul, bf16 x input.

Set V2DEBUG=1 to add intermediate dumps (core-0 debugging).
"""
import os
import numpy as np
import ml_dtypes
import concourse.bass as bass
import concourse.bacc as bacc
import concourse.mybir as mybir
from concourse import tile
from concourse.masks import make_identity
from concourse.bass_utils import run_bass_kernel_spmd

F32 = mybir.dt.float32
BF16 = mybir.dt.bfloat16
AF = mybir.ActivationFunctionType
OP = mybir.AluOpType

B, HH, WW, C = 4, 64, 64, 768
WIN, NH, HD = 14, 12, 64
S = WIN
N = S * S
GRID = 5
NWIN_TOT = B * GRID * GRID
NCORE = 8
NW = 13
NTOK = NW * N
NT = 2 * N
NPAIR = 7
NCH = C // 128
DFF = 4 * C
SCALE = HD ** -0.5
EPS = 1e-6
P34_TILES = [392] * 6 + [196]
DEBUG = bool(os.environ.get("V2DEBUG"))

_CACHE = {}


def _build():
    nc = bacc.Bacc("TRN2", target_bir_lowering=False, debug=False)
    dt_in = {}

    def din(name, shape, dt=F32):
        dt_in[name] = nc.dram_tensor(name, shape, dt, kind="ExternalInput")
        return dt_in[name]

    xT_d = din("xT", [C, NTOK], BF16)
    rgbT_d = din("rgbT", [C, NTOK], BF16)
    mask_d = din("mask", [1, NTOK], BF16)
    negbig_d = din("negbig", [1, NTOK], BF16)
    imgmask_d = din("imgmask", [4, NW])
    imgneg_d = din("imgneg", [4, NW])
    imgsel_d = din("imgsel", [4, NTOK], BF16)
    qkwT_d = din("qkwT", [C, 2 * C], BF16)
    qkb_d = din("qkb", [2 * C])
    vwT_d = din("vwT", [C, C], BF16)
    projwT_d = din("projwT", [C, C], BF16)
    projb_d = din("projb", [C])
    tab_d = din("tab", [HD, 54])
    fc1wT_d = din("fc1wT", [C, DFF], BF16)
    fc1b_d = din("fc1b", [DFF])
    fc2wT_d = din("fc2wT", [DFF, C], BF16)
    fc2b_d = din("fc2b", [C])
    sw1wT_d = din("sw1wT", [2 * C, C], BF16)
    sw1b_d = din("sw1b", [C])
    sw2wT_d = din("sw2wT", [C, 2], BF16)
    sw2b_d = din("sw2b", [2])
    cw1wTs_d = din("cw1wTs", [4 * C, 4 * C // NCORE], BF16)
    cw1bs_d = din("cw1bs", [4 * C // NCORE])
    cw2rT_d = din("cw2rT", [4 * C // NCORE, 2 * C], BF16)
    cw2bf_d = din("cw2bf", [2 * C])
    cwaug_d = din("cwaug", [4, NCH * 128], BF16)
    vones_d = din("vones", [98, NH], BF16)
    out1_d = nc.dram_tensor("out1T", [C, NTOK], BF16, kind="ExternalOutput")
    out2_d = nc.dram_tensor("out2T", [C, NTOK], BF16, kind="ExternalOutput")
    if DEBUG:
        dbg = {
            "dlnx": nc.dram_tensor("dlnx", [128, NCH * NT], BF16, kind="ExternalOutput"),
            "deh": nc.dram_tensor("deh", [98, NH * 14], BF16, kind="ExternalOutput"),
            "dew": nc.dram_tensor("dew", [98, NH * 14], BF16, kind="ExternalOutput"),
            "dP": nc.dram_tensor("dP", [98, 2 * 196], BF16, kind="ExternalOutput"),
            "dz": nc.dram_tensor("dz", [1, 2 * 196], F32, kind="ExternalOutput"),
            "dzbb": nc.dram_tensor("dzbb", [64, 2 * 196], F32, kind="ExternalOutput"),
            "dattn": nc.dram_tensor("dattn", [128, NCH * NT], BF16, kind="ExternalOutput"),
            "dx2a": nc.dram_tensor("dx2a", [C, NTOK], BF16, kind="ExternalOutput"),
            "dx2b": nc.dram_tensor("dx2b", [128, NCH * NT], BF16, kind="ExternalOutput"),
            "ds01": nc.dram_tensor("ds01", [2, NTOK], BF16, kind="ExternalOutput"),
            "dyf": nc.dram_tensor("dyf", [128, 12 * 4], F32, kind="ExternalOutput"),
        }

    core_ids = list(range(NCORE))
    r6 = lambda ap: ap.rearrange("(c p) n -> p c n", p=128)
    rcol = lambda ap: ap.rearrange("(c p) -> p c", p=128)

    with tile.TileContext(nc) as tc:
      with tc.tile_pool(name="dram", bufs=1, space="DRAM") as dramp, \
           tc.tile_pool(name="fbp", bufs=6, space="DRAM") as fbp, \
           tc.tile_pool(name="cst", bufs=1) as cp, \
           tc.tile_pool(name="pers", bufs=1) as pers, \
           tc.tile_pool(name="stg", bufs=2) as stg:
        x2a_d = dramp.tile([C, NTOK], BF16)
        csum_in = dramp.tile([128, 48], F32)
        csum_out = dramp.tile([128, 48], F32, addr_space="Shared")
        cmax_in = dramp.tile([128, 48], F32)
        cmax_out = dramp.tile([128, 48], F32, addr_space="Shared")
        z2_in = dramp.tile([2 * C, 4], F32)
        z2_out = dramp.tile([2 * C, 4], F32, addr_space="Shared")

        def load_rows(src, n=C):
            t = cp.tile([128, n // 128], F32, name="rows_" + src.tensor.name)
            nc.sync.dma_start(t[:], rcol(src))
            return t

        qkb_t = load_rows(qkb_d[:], 2 * C)
        projb_t = load_rows(projb_d[:])
        fc1b_t = load_rows(fc1b_d[:], DFF)
        fc2b_t = load_rows(fc2b_d[:])
        sw1b_t = load_rows(sw1b_d[:])
        sw2b_t = cp.tile([2, 1], F32)
        nc.sync.dma_start(sw2b_t[:, 0], sw2b_d[:])
        cw1bs_t = load_rows(cw1bs_d[:], 4 * C // NCORE)
        cw2bf_t = load_rows(cw2bf_d[:], 2 * C)
        tabf = stg.tile([128, 2, 54], F32, tag="st1")
        nc.any.memset(tabf[:], 0.0)
        nc.sync.dma_start(tabf[0:64, 0, :], tab_d[:])
        nc.sync.dma_start(tabf[64:128, 1, :], tab_d[:])
        ones_f = stg.tile([128, 1], F32, tag="st1")
        nc.any.memset(ones_f[:], 1.0)
        ones_b = cp.tile([128, 1], BF16)
        nc.vector.tensor_copy(ones_b[:], ones_f[:])

        def load_bf(pool_, shape3, src_ap, nm):
            r = pool_.tile(shape3, BF16, name="w_" + nm)
            nc.sync.dma_start(r[:], src_ap)
            return r

        # ========== PHASE 0: rgb FRM window stats (overlaps attention) ======
        negrow = stg.tile([1, NTOK], BF16, tag="rgb0", name="negrow")
        nc.sync.dma_start(negrow[:], negbig_d[:])
        neg_full = pers.tile([128, NTOK], BF16)
        nc.gpsimd.partition_broadcast(neg_full[:], negrow[:])
        mskrow = stg.tile([1, NTOK], BF16, tag="rgb0", name="mskrow")
        nc.sync.dma_start(mskrow[:], mask_d[:])
        msk_full = pers.tile([128, NTOK], BF16)
        nc.gpsimd.partition_broadcast(msk_full[:], mskrow[:])

        # ==================== PHASE 1: attention ====================
        with tc.tile_pool(name="w1", bufs=1) as wp1, \
             tc.tile_pool(name="p1", bufs=2) as p1, \
             tc.tile_pool(name="p1s", bufs=2) as p1s, \
             tc.tile_pool(name="p1p", bufs=10) as p1p, \
             tc.tile_pool(name="p1f", bufs=2) as p1f, \
             tc.tile_pool(name="gen", bufs=2, space="PSUM") as psg, \
             tc.tile_pool(name="scp", bufs=2, space="PSUM") as pscp, \
             tc.tile_pool(name="psa", bufs=2, space="PSUM") as psa, \
             tc.tile_pool(name="pst", bufs=2, space="PSUM") as pst:
            qkwT = load_bf(wp1, [128, NCH, 2 * C], r6(qkwT_d[:]), "qk")
            vwT = load_bf(wp1, [128, NCH, C], r6(vwT_d[:]), "v")
            projwT = load_bf(wp1, [128, NCH, C], r6(projwT_d[:]), "pj")
            tab2b = wp1.tile([128, 2, 54], BF16)
            nc.vector.tensor_copy(tab2b[:], tabf[:])
            idstage = wp1.tile([128, 128], F32, tag="idst", bufs=1)
            make_identity(nc, idstage)
            ident98b = wp1.tile([98, 98], BF16)
            nc.vector.tensor_copy(ident98b[:], idstage[0:98, 0:98])

            kz0 = p1.tile([128, NH, NT], BF16, tag="kz0", bufs=1)
            zst = wp1.tile([64, NT], BF16, tag="wstage", bufs=1)
            nc.any.memset(zst[:], 0.0)
            for j in range(NH // 2):
                nc.vector.tensor_copy(kz0[64:128, 2 * j, :], zst[:])
                nc.vector.tensor_copy(kz0[0:64, 2 * j + 1, :], zst[:])

            for blk in range(NPAIR):
                degen = blk == NPAIR - 1
                wr = [0] if degen else [0, 1]
                c0 = blk * NT
                cols = slice(c0, c0 + NT) if not degen else None
                xt = p1.tile([128, NCH, NT], BF16, tag="xt")
                if degen:
                    nc.sync.dma_start(xt[:, :, 0:N], r6(xT_d[:])[:, :, 2352:2548])
                    nc.sync.dma_start(xt[:, :, N:NT], r6(xT_d[:])[:, :, 2352:2548])
                else:
                    nc.sync.dma_start(xt[:], r6(xT_d[:])[:, :, cols])

                s1 = psg.tile([128, 512], F32, tag="gen", name=f"s1_{blk}")
                s2 = psg.tile([128, 512], F32, tag="gen", name=f"s2_{blk}")
                for c in range(NCH):
                    nc.tensor.matmul(s1[0:1, 0:NT], ones_b[:], xt[:, c, :],
                                     start=(c == 0), stop=(c == NCH - 1))
                for c in range(NCH):
                    xsqc = p1s.tile([128, NT], BF16, tag="xsqc",
                                    name=f"xsq_{blk}_{c}")
                    nc.vector.tensor_mul(xsqc[:], xt[:, c, :], xt[:, c, :])
                    nc.tensor.matmul(s2[0:1, 0:NT], ones_b[:], xsqc[:],
                                     start=(c == 0), stop=(c == NCH - 1))
                rws = p1f.tile([1, 3, NT], F32, tag="rws", name=f"rw1_{blk}")
                nc.vector.tensor_scalar_mul(rws[:, 0, :], s1[0:1, 0:NT], 1.0 / C)
                nc.vector.tensor_mul(rws[:, 1, :], rws[:, 0, :], rws[:, 0, :])
                nc.vector.tensor_scalar_mul(rws[:, 2, :], s2[0:1, 0:NT], 1.0 / C)
                nc.vector.tensor_sub(rws[:, 1, :], rws[:, 2, :], rws[:, 1, :])
                nc.vector.tensor_scalar_add(rws[:, 1, :], rws[:, 1, :], EPS)
                nc.scalar.activation(rws[:, 2, :], rws[:, 1, :], AF.Sqrt)
                nc.vector.reciprocal(rws[:, 1, :], rws[:, 2, :])
                rwb = p1f.tile([1, 2, NT], BF16, tag="rwb", name=f"rb1_{blk}")
                nc.vector.tensor_copy(rwb[:, 0, :], rws[:, 0, :])
                nc.vector.tensor_copy(rwb[:, 1, :], rws[:, 1, :])
                m_b = p1f.tile([128, NT], BF16, tag="m_b")
                nc.gpsimd.partition_broadcast(m_b[:], rwb[:, 0, :])
                inv_b = p1f.tile([128, NT], BF16, tag="inv_b")
                nc.gpsimd.partition_broadcast(inv_b[:], rwb[:, 1, :])
                lnx_r = p1.tile([128, NCH, NT], BF16, tag="lnx_r")
                for c in range(NCH):
                    u = p1s.tile([128, NT], BF16, tag="u", name=f"u_{blk}_{c}")
                    nc.vector.tensor_sub(u[:], xt[:, c, :], m_b[:])
                    nc.vector.tensor_mul(lnx_r[:, c, :], u[:], inv_b[:])
                if DEBUG and blk == 0:
                    nc.sync.dma_start(
                        dbg["dlnx"][:].rearrange("p (c n) -> p c n", n=NT),
                        lnx_r[:])

                qT = p1.tile([128, NCH, NT], BF16, tag="qT")
                kzt = kz0
                for mc in range(12):
                    ps = psg.tile([128, 512], F32, tag="gen", name=f"qk_{blk}_{mc}")
                    for kc in range(NCH):
                        nc.tensor.matmul(ps[0:128, 0:NT],
                                         qkwT[:, kc, mc * 128:(mc + 1) * 128],
                                         lnx_r[:, kc, :], start=(kc == 0),
                                         stop=(kc == NCH - 1))
                    if mc < NCH:
                        nc.vector.tensor_scalar(qT[:, mc, :], ps[0:128, 0:NT],
                                                SCALE, qkb_t[:, mc:mc + 1],
                                                OP.mult, OP.add)
                    else:
                        j = mc - NCH
                        nc.vector.tensor_scalar_add(kzt[0:64, 2 * j, :],
                                                    ps[0:64, 0:NT],
                                                    qkb_t[0:64, mc:mc + 1])
                        nc.vector.tensor_scalar_add(kzt[64:128, 2 * j + 1, :],
                                                    ps[64:128, 0:NT],
                                                    qkb_t[64:128, mc:mc + 1])

                v_sb = {}
                for w in wr:
                    for tch in range(2):
                        t0 = w * N + tch * 98
                        vt = p1.tile([98, NH, 65], BF16, tag=f"v{w}{tch}",
                                     name=f"v{w}{tch}_{blk}")
                        ps_a = psg.tile([128, 512], F32, tag="gen",
                                        name=f"va_{blk}_{w}_{tch}")
                        ps_b = psg.tile([128, 512], F32, tag="gen",
                                        name=f"vb_{blk}_{w}_{tch}")
                        for kc in range(NCH):
                            nc.tensor.matmul(ps_a[0:98, 0:512],
                                             lnx_r[:, kc, t0:t0 + 98],
                                             vwT[:, kc, 0:512], start=(kc == 0),
                                             stop=(kc == NCH - 1))
                        for kc in range(NCH):
                            nc.tensor.matmul(ps_b[0:98, 0:256],
                                             lnx_r[:, kc, t0:t0 + 98],
                                             vwT[:, kc, 512:768], start=(kc == 0),
                                             stop=(kc == NCH - 1))
                        nc.scalar.copy(vt[:, 0:8, 0:64],
                                       ps_a[0:98, 0:512].rearrange(
                                           "p (h d) -> p h d", d=64))
                        nc.scalar.copy(vt[:, 8:12, 0:64],
                                       ps_b[0:98, 0:256].rearrange(
                                           "p (h d) -> p h d", d=64))
                        nc.sync.dma_start(vt[:, :, 64], vones_d[:])
                        v_sb[(w, tch)] = vt

                e_h, e_w = {}, {}
                for w in wr:
                    gh2 = p1f.tile([98, NH, 2, 14, 1], BF16, tag=f"gh{w}",
                                   name=f"gh{w}_{blk}")
                    gw2 = p1f.tile([98, NH, 2, 1, 14], BF16, tag=f"gw{w}",
                                   name=f"gw{w}_{blk}")
                    for qc in range(2):
                        ps1 = psg.tile([128, 512], F32, tag="gen",
                                       name=f"f1_{blk}_{w}_{qc}")
                        ps2 = psg.tile([128, 512], F32, tag="gen",
                                       name=f"f2_{blk}_{w}_{qc}")
                        for h in range(NH):
                            dst = (ps1[0:98, h * 54:h * 54 + 54] if h < 9 else
                                   ps2[0:98, (h - 9) * 54:(h - 9) * 54 + 54])
                            lhsT = qT[:, h // 2, w * N + qc * 98: w * N + qc * 98 + 98]
                            nc.tensor.matmul(dst, lhsT, tab2b[:, h % 2, :],
                                             start=True, stop=True)
                        f = p1s.tile([98, NH, 54], BF16, tag="ftile",
                                     name=f"f{w}{qc}_{blk}")
                        nc.scalar.copy(f[:, 0:9, :],
                                       ps1[0:98, 0:486].rearrange(
                                           "p (h t) -> p h t", t=54))
                        nc.scalar.copy(f[:, 9:12, :],
                                       ps2[0:98, 0:162].rearrange(
                                           "p (h t) -> p h t", t=54))
                        fd = fbp.tile([98, NH, 54], BF16, tag="fb",
                                      name=f"fd{w}{qc}_{blk}")
                        nc.scalar.dma_start(fd[:], f[:])
                        for hb in range(7):
                            h_glob = qc * 7 + hb
                            nc.sync.dma_start(
                                gh2[hb * 14:(hb + 1) * 14, :, qc, :, 0],
                                f[hb * 14:(hb + 1) * 14, :, 13 - h_glob:27 - h_glob])
                            src = bass.AP(fd[:].tensor, hb * 14 * 648 + 40,
                                          [[647, 14], [54, NH], [1, 14]])
                            nc.scalar.dma_start(
                                gw2[hb * 14:(hb + 1) * 14, :, qc, 0, :], src)
                    eh2 = p1f.tile([98, NH, 2, 14, 1], BF16, tag=f"eh{w}",
                                   name=f"eh{w}_{blk}")
                    ew2 = p1f.tile([98, NH, 2, 1, 14], BF16, tag=f"ew{w}",
                                   name=f"ew{w}_{blk}")
                    nc.scalar.activation(eh2[:], gh2[:], AF.Exp)
                    nc.scalar.activation(ew2[:], gw2[:], AF.Exp)
                    e_h[w], e_w[w] = eh2, ew2
                    if DEBUG and blk == 0 and w == 0:
                        nc.sync.dma_start(
                            dbg["deh"][:].rearrange("p (a b) -> p a b", b=14),
                            eh2[:, :, 0, :, 0])
                        nc.sync.dma_start(
                            dbg["dew"][:].rearrange("p (a b) -> p a b", b=14),
                            ew2[:, :, 0, 0, :])

                attn_outT = p1.tile([128, NCH, NT], BF16, tag="attn_outT")
                for h in range(NH):
                    psA_w = {}
                    zrow = p1f.tile([1, 2, 196], F32, tag="zrow",
                                    name=f"zr_{blk}_{h}")
                    zsb = p1f.tile([1, 2, 196], F32, tag="zsb",
                                   name=f"zsb_{blk}_{h}")
                    for w in wr:
                        P = p1p.tile([98, 2, 196], BF16, tag="P",
                                     name=f"P_{blk}_{h}_{w}")
                        sps2 = pscp.tile([98, 2, 196], F32, tag="sc",
                                         name=f"sc_{blk}_{h}_{w}")
                        for qc in range(2):
                            lhsT = qT[:, h // 2,
                                      w * N + qc * 98: w * N + qc * 98 + 98]
                            nc.tensor.matmul(sps2[:, qc, :], lhsT,
                                             kzt[:, h, w * N:w * N + 196],
                                             start=True, stop=True)
                        nc.scalar.activation(P[:], sps2[:], AF.Exp)
                        P4v = P[:].rearrange("p a (b c) -> p a b c", c=14)
                        nc.vector.tensor_mul(
                            P4v, P4v,
                            e_h[w][:, h, :, :, :].broadcast_to([98, 2, 14, 14]))
                        nc.vector.tensor_mul(
                            P4v, P4v,
                            e_w[w][:, h, :, :, :].broadcast_to([98, 2, 14, 14]))
                        if DEBUG and blk == 0 and h == 0 and w == 0:
                            nc.sync.dma_start(
                                dbg["dP"][:].rearrange("p (a b) -> p a b", b=196),
                                P[:])
                        PT_sb = []
                        for kc in range(2):
                            ptp = pst.tile([98, 2, 98], BF16, tag="pt",
                                           name=f"pt_{blk}_{h}_{w}_{kc}")
                            for qc in range(2):
                                nc.tensor.transpose(
                                    ptp[:, qc, :],
                                    P[:, qc, kc * 98:kc * 98 + 98], ident98b[:])
                            sb = p1p.tile([98, 2, 98], BF16, tag="PTsb",
                                          name=f"ptsb_{blk}_{h}_{w}_{kc}")
                            nc.scalar.copy(sb[:], ptp[:])
                            PT_sb.append(sb)
                        psA = psa.tile([65, 196], F32, tag="psAV",
                                       name=f"av_{blk}_{h}_{w}")
                        for kc in range(2):
                            nc.tensor.matmul(
                                psA[:],
                                v_sb[(w, kc)][:, h, :],
                                PT_sb[kc][:].rearrange("p a b -> p (a b)"),
                                start=(kc == 0), stop=(kc == 1))
                        # stage z to partition-0 SBUF: the custom-DVE
                        # reciprocal misreads PSUM at partition offset 64
                        nc.scalar.copy(zsb[:, w, :], psA[64:65, :])
                        psA_w[w] = psA
                    zbb = p1f.tile([64, 2, 196], F32, tag="zbb",
                                   name=f"zbb_{blk}_{h}")
                    if degen:
                        nc.vector.reciprocal_approx_fast(zrow[:, 0, :],
                                                         zsb[:, 0, :])
                        nc.gpsimd.partition_broadcast(zbb[:, 0, :], zrow[:, 0, :])
                    else:
                        nc.vector.reciprocal_approx_fast(
                            zrow[:].rearrange("p a b -> p (a b)"),
                            zsb[:].rearrange("p a b -> p (a b)"))
                        nc.gpsimd.partition_broadcast(
                            zbb[:].rearrange("p a b -> p (a b)"),
                            zrow[:].rearrange("p a b -> p (a b)"))
                    if DEBUG and blk == 0 and h == 0:
                        nc.sync.dma_start(
                            dbg["dz"][:].rearrange("p (a b) -> p a b", b=196),
                            zrow[:])
                        nc.sync.dma_start(
                            dbg["dzbb"][:].rearrange("p (a b) -> p a b", b=196),
                            zbb[:])
                    for w in wr:
                        nc.vector.tensor_mul(
                            attn_outT[(h % 2) * 64:(h % 2) * 64 + 64, h // 2,
                                      w * N:w * N + 196],
                            psA_w[w][0:64, :], zbb[:, w, :])
                if DEBUG and blk == 0:
                    nc.sync.dma_start(
                        dbg["dattn"][:].rearrange("p (c n) -> p c n", n=NT),
                        attn_outT[:])

                nout = N if degen else NT
                outc = p1s.tile([128, NCH, NT], BF16, tag="outc")
                for mc in range(NCH):
                    ps = psg.tile([128, 512], F32, tag="gen", name=f"pj_{blk}_{mc}")
                    for kc in range(NCH):
                        nc.tensor.matmul(ps[0:128, 0:nout],
                                         projwT[:, kc, mc * 128:(mc + 1) * 128],
                                         attn_outT[:, kc, 0:nout], start=(kc == 0),
                                         stop=(kc == NCH - 1))
                    nc.vector.scalar_tensor_tensor(
                        outc[:, mc, 0:nout], ps[0:128, 0:nout],
                        projb_t[:, mc:mc + 1], xt[:, mc, 0:nout],
                        OP.add, OP.add)
                dstc = (r6(x2a_d[:])[:, :, 2352:2548] if degen
                        else r6(x2a_d[:])[:, :, cols])
                nc.sync.dma_start(dstc, outc[:, :, 0:nout])

        W_s1 = pers.tile([128, NCH, NW, 1], F32)
        W_m1 = pers.tile([128, NCH, NW, 1], F32)
        col0 = 0
        for ti, tl in enumerate(P34_TILES):
            nwt = tl // N
            cs = slice(col0, col0 + tl)
            rgb0 = stg.tile([128, NCH, tl], BF16, tag="rgb0", name=f"rgb0_{ti}")
            nc.sync.dma_start(rgb0[:], r6(rgbT_d[:])[:, :, cs])
            for c in range(NCH):
                x1n = stg.tile([128, tl], BF16, tag="x1n", name=f"x1n_{ti}_{c}")
                nc.vector.tensor_add(x1n[:], rgb0[:, c, :], neg_full[:, cs])
                nc.vector.tensor_reduce(
                    W_s1[:, c, 2 * ti:2 * ti + nwt, :],
                    rgb0[:, c, :].rearrange("p (w n) -> p w n", n=N),
                    mybir.AxisListType.X, OP.add)
                nc.vector.tensor_reduce(
                    W_m1[:, c, 2 * ti:2 * ti + nwt, :],
                    x1n[:].rearrange("p (w n) -> p w n", n=N),
                    mybir.AxisListType.X, OP.max)
            col0 += tl

        if DEBUG:
            nc.sync.dma_start(dbg["dx2a"][:], x2a_d[:])

        # ====== PHASE 2: MLP + FRM stats ======
        with tc.tile_pool(name="p3a", bufs=1) as p3a:
          x2breg = p3a.tile([128, NCH, NTOK], BF16, name="x2breg")
          W_s2 = p3a.tile([128, NCH, NW, 1], F32)
          W_m2 = p3a.tile([128, NCH, NW, 1], F32)
          s01_all = p3a.tile([2, NTOK], BF16)

          with tc.tile_pool(name="w2", bufs=1) as wp2, \
               tc.tile_pool(name="p2", bufs=2) as p2, \
               tc.tile_pool(name="p2f", bufs=2) as p2f, \
               tc.tile_pool(name="ps1p", bufs=2, space="PSUM") as ps1p, \
               tc.tile_pool(name="ps2p", bufs=6, space="PSUM") as ps2p:
            fc1wT = load_bf(wp2, [128, NCH, DFF], r6(fc1wT_d[:]), "fc1")
            fc2wT = load_bf(wp2, [128, DFF // 128, C], r6(fc2wT_d[:]), "fc2")
            col0 = 0
            for ti, tl in enumerate(P34_TILES):
                nwt = tl // N
                cs = slice(col0, col0 + tl)
                xa = p2.tile([128, NCH, tl], BF16, tag="xa", name=f"xa_{col0}")
                nc.sync.dma_start(xa[:], r6(x2a_d[:])[:, :, cs])
                s1 = ps1p.tile([128, tl], F32, tag="ps1", name=f"s1_{col0}")
                s2 = ps1p.tile([128, tl], F32, tag="ps1", name=f"s2_{col0}")
                for c in range(NCH):
                    nc.tensor.matmul(s1[0:1, :], ones_b[:], xa[:, c, :],
                                     start=(c == 0), stop=(c == NCH - 1))
                for c in range(NCH):
                    xsqc = p2.tile([128, tl], BF16, tag="xsqc",
                                   name=f"xsq2_{col0}_{c}")
                    nc.vector.tensor_mul(xsqc[:], xa[:, c, :], xa[:, c, :])
                    nc.tensor.matmul(s2[0:1, :], ones_b[:], xsqc[:],
                                     start=(c == 0), stop=(c == NCH - 1))
                rws = p2f.tile([1, 3, tl], F32, tag="rws", name=f"rw_{col0}",
                               bufs=1)
                nc.vector.tensor_scalar_mul(rws[:, 0, :], s1[0:1, :], 1.0 / C)
                nc.vector.tensor_mul(rws[:, 1, :], rws[:, 0, :], rws[:, 0, :])
                nc.vector.tensor_scalar_mul(rws[:, 2, :], s2[0:1, :], 1.0 / C)
                nc.vector.tensor_sub(rws[:, 1, :], rws[:, 2, :], rws[:, 1, :])
                nc.vector.tensor_scalar_add(rws[:, 1, :], rws[:, 1, :], EPS)
                nc.scalar.activation(rws[:, 2, :], rws[:, 1, :], AF.Sqrt)
                nc.vector.reciprocal(rws[:, 1, :], rws[:, 2, :])
                rwb = p2f.tile([1, 2, tl], BF16, tag="rwb", name=f"rb_{col0}")
                nc.vector.tensor_copy(rwb[:, 0, :], rws[:, 0, :])
                nc.vector.tensor_copy(rwb[:, 1, :], rws[:, 1, :])
                m_b = p2f.tile([128, tl], BF16, tag="m_b", name=f"mb_{col0}")
                nc.gpsimd.partition_broadcast(m_b[:], rwb[:, 0, :])
                inv_b = p2f.tile([128, tl], BF16, tag="inv_b", name=f"ib_{col0}")
                nc.gpsimd.partition_broadcast(inv_b[:], rwb[:, 1, :])
                ln2_r = p2.tile([128, NCH, tl], BF16, tag="ln2_r", name=f"l2_{col0}")
                for c in range(NCH):
                    u = p2.tile([128, tl], BF16, tag="u", name=f"u_{col0}_{c}")
                    nc.vector.tensor_sub(u[:], xa[:, c, :], m_b[:])
                    nc.vector.tensor_mul(ln2_r[:, c, :], u[:], inv_b[:])
                psum2 = [ps2p.tile([128, tl], F32, tag="ps2", name=f"ps2_{col0}_{m}")
                         for m in range(NCH)]
                for kc in range(DFF // 128):
                    ps1 = ps1p.tile([128, tl], F32, tag="ps1", name=f"ps1_{col0}_{kc}")
                    for c in range(NCH):
                        nc.tensor.matmul(ps1[:], fc1wT[:, c, kc * 128:(kc + 1) * 128],
                                         ln2_r[:, c, :], start=(c == 0),
                                         stop=(c == NCH - 1))
                    h1 = p2.tile([128, tl], BF16, tag="h1", name=f"h1_{col0}_{kc}")
                    nc.scalar.activation(h1[:], ps1[:], AF.Gelu,
                                         bias=fc1b_t[:, kc:kc + 1], scale=1.0)
                    for mc in range(NCH):
                        nc.tensor.matmul(psum2[mc][:],
                                         fc2wT[:, kc, mc * 128:(mc + 1) * 128],
                                         h1[:], start=(kc == 0),
                                         stop=(kc == DFF // 128 - 1))
                for mc in range(NCH):
                    x2f = p2.tile([128, tl], BF16, tag="x2f", name=f"x2f_{col0}_{mc}")
                    nc.vector.scalar_tensor_tensor(
                        x2f[:], psum2[mc][:], fc2b_t[:, mc:mc + 1],
                        xa[:, mc, :], OP.add, OP.add)
                    nc.vector.tensor_mul(x2breg[:, mc, cs], x2f[:],
                                         msk_full[:, cs])
                    xmn = p2.tile([128, tl], BF16, tag="xmn", name=f"xn_{col0}_{mc}")
                    nc.vector.tensor_add(xmn[:], x2breg[:, mc, cs],
                                         neg_full[:, cs])
                    nc.vector.tensor_reduce(
                        W_s2[:, mc, 2 * ti:2 * ti + nwt, :],
                        x2breg[:, mc, cs].rearrange("p (w n) -> p w n", n=N),
                        mybir.AxisListType.X, OP.add)
                    nc.vector.tensor_reduce(
                        W_m2[:, mc, 2 * ti:2 * ti + nwt, :],
                        xmn[:].rearrange("p (w n) -> p w n", n=N),
                        mybir.AxisListType.X, OP.max)
                col0 += tl
          if DEBUG:
              nc.sync.dma_start(
                  dbg["dx2b"][:].rearrange("p (c n) -> p c n", n=NT),
                  x2breg[:, :, 0:NT])

          # rgb resident for sw path + final combine (space freed by wp2)
          x1breg = p3a.tile([128, NCH, NTOK], BF16, name="x1breg")
          nc.sync.dma_start(x1breg[:], r6(rgbT_d[:]))

          # ====== image stats combine + collectives + channel MLP ======
          with tc.tile_pool(name="p3", bufs=2) as p3, \
               tc.tile_pool(name="w3", bufs=1) as wp3, \
               tc.tile_pool(name="sps", bufs=1, space="PSUM") as sps:
              ident128 = p3a.tile([128, 128], F32)
              idstage = p3.tile([128, 128], F32, tag="idst")
              make_identity(nc, idstage)
              nc.vector.tensor_copy(ident128[:], idstage[:])
              imb = p3a.tile([128, 4, NW], F32)
              inb = p3a.tile([128, 4, NW], F32)
              for i in range(4):
                  r = stg.tile([1, NW], F32, tag="imrow")
                  nc.sync.dma_start(r[:], imgmask_d[i:i + 1, :])
                  nc.gpsimd.partition_broadcast(imb[:, i, :], r[:])
                  r2 = stg.tile([1, NW], F32, tag="imrow")
                  nc.sync.dma_start(r2[:], imgneg_d[i:i + 1, :])
                  nc.gpsimd.partition_broadcast(inb[:, i, :], r2[:])
              stat_s = p3a.tile([128, NCH, 2, 4], F32)
              stat_m = p3a.tile([128, NCH, 2, 4], F32)
              for k, Wt in ((0, W_s1), (1, W_s2)):
                  for i in range(4):
                      t = p3.tile([128, NCH, NW], F32, tag="cmb", name=f"cmb_{k}_{i}")
                      nc.vector.tensor_mul(t[:], Wt[:, :, :, 0],
                                           imb[:, i, None, :].broadcast_to(
                                               [128, NCH, NW]))
                      nc.vector.tensor_reduce(stat_s[:, :, k, i:i + 1], t[:],
                                              mybir.AxisListType.X, OP.add)
              for k, Wt in ((0, W_m1), (1, W_m2)):
                  for i in range(4):
                      t = p3.tile([128, NCH, NW], F32, tag="cmb", name=f"cmbm_{k}_{i}")
                      nc.vector.tensor_mul(t[:], Wt[:, :, :, 0],
                                           imb[:, i, None, :].broadcast_to(
                                               [128, NCH, NW]))
                      nc.vector.tensor_add(t[:], t[:],
                                           inb[:, i, None, :].broadcast_to(
                                               [128, NCH, NW]))
                      nc.vector.tensor_reduce(stat_m[:, :, k, i:i + 1], t[:],
                                              mybir.AxisListType.X, OP.max)
              nc.sync.dma_start(csum_in[:], stat_s[:].rearrange("p a b c -> p (a b c)"))
              nc.sync.dma_start(cmax_in[:], stat_m[:].rearrange("p a b c -> p (a b c)"))
              nc.gpsimd.collective_compute("AllReduce", OP.add,
                                           replica_groups=[core_ids],
                                           ins=[csum_in[:]], outs=[csum_out[:]])
              nc.gpsimd.collective_compute("AllReduce", OP.max,
                                           replica_groups=[core_ids],
                                           ins=[cmax_in[:]], outs=[cmax_out[:]])

              sw1wT = load_bf(wp3, [128, 2 * NCH, C], sw1wT_d[:].rearrange(
                  "(c p) m -> p c m", p=128), "sw1")
              sw2wT = load_bf(wp3, [128, NCH, 2], r6(sw2wT_d[:]), "sw2")
              with tc.tile_pool(name="p3b", bufs=2) as p3b, \
                   tc.tile_pool(name="zps", bufs=6, space="PSUM") as zps:
                col0 = 0
                for ti, tl in enumerate(P34_TILES):
                    cs = slice(col0, col0 + tl)
                    zpsl = [zps.tile([128, tl], F32, tag="zp", name=f"zp_{col0}_{m}")
                            for m in range(NCH)]
                    for mc in range(NCH):
                        for kc in range(2 * NCH):
                            rhs = (x1breg[:, kc, cs] if kc < NCH
                                   else x2breg[:, kc - NCH, cs])
                            nc.tensor.matmul(zpsl[mc][:],
                                             sw1wT[:, kc, mc * 128:(mc + 1) * 128],
                                             rhs, start=(kc == 0),
                                             stop=(kc == 2 * NCH - 1))
                    z_r = p3b.tile([128, NCH, tl], BF16, tag="z_r", name=f"zr_{col0}")
                    for mc in range(NCH):
                        nc.vector.tensor_scalar(z_r[:, mc, :], zpsl[mc][:],
                                                sw1b_t[:, mc:mc + 1], 0.0,
                                                OP.add, OP.max)
                    sps_t = sps.tile([2, tl], F32, tag="sp", name=f"sp_{col0}")
                    for kc in range(NCH):
                        nc.tensor.matmul(sps_t[:], sw2wT[:, kc, :], z_r[:, kc, :],
                                         start=(kc == 0), stop=(kc == NCH - 1))
                    nc.vector.tensor_scalar_add(s01_all[:, cs], sps_t[:], sw2b_t[:])
                    col0 += tl
                nc.scalar.activation(s01_all[:], s01_all[:], AF.Sigmoid)
                if DEBUG:
                    nc.sync.dma_start(dbg["ds01"][:], s01_all[:])

              ycat_f = p3a.tile([128, 24, 4], F32)
              cso4 = csum_out[:].rearrange("p (c k i) -> p c k i", k=2, i=4)
              cmo4 = cmax_out[:].rearrange("p (c k i) -> p c k i", k=2, i=4)
              nc.sync.dma_start(ycat_f[:, 0:6, :], cso4[:, :, 0, :])
              nc.sync.dma_start(ycat_f[:, 6:12, :], cso4[:, :, 1, :])
              nc.sync.dma_start(ycat_f[:, 12:18, :], cmo4[:, :, 0, :])
              nc.sync.dma_start(ycat_f[:, 18:24, :], cmo4[:, :, 1, :])
              ycat_r = p3a.tile([128, 24, 4], BF16)
              nc.vector.tensor_copy(ycat_r[:], ycat_f[:])
              cw1wTs = load_bf(wp3, [128, 24, 4 * C // NCORE],
                               cw1wTs_d[:].rearrange("(c p) m -> p c m", p=128), "cw1")
              z1sb = p3a.tile([128, 3, 4], F32)
              for mc in range(3):
                  ps = sps.tile([128, 4], F32, tag="sp", name=f"z1ps_{mc}")
                  for kc in range(24):
                      nc.tensor.matmul(ps[:], cw1wTs[:, kc, mc * 128:(mc + 1) * 128],
                                       ycat_r[:, kc, :], start=(kc == 0), stop=(kc == 23))
                  nc.scalar.activation(z1sb[:, mc, :], ps[:], AF.Relu,
                                       bias=cw1bs_t[:, mc:mc + 1], scale=1.0)
              z1r = p3a.tile([128, 3, 4], BF16)
              nc.vector.tensor_copy(z1r[:], z1sb[:])
              cw2r = load_bf(wp3, [128, 3, 2 * C],
                             cw2rT_d[:].rearrange("(c p) m -> p c m", p=128),
                             "cw2")
              z2p = p3a.tile([128, 12, 4], F32)
              for mc in range(12):
                  ps = sps.tile([128, 4], F32, tag="sp", name=f"z2ps_{mc}")
                  for kc in range(3):
                      nc.tensor.matmul(ps[:], cw2r[:, kc, mc * 128:(mc + 1) * 128],
                                       z1r[:, kc, :], start=(kc == 0),
                                       stop=(kc == 2))
                  nc.scalar.copy(z2p[:, mc, :], ps[:])
              nc.sync.dma_start(z2_in[:].rearrange("(m p) f -> p m f", p=128),
                                z2p[:])
              nc.gpsimd.collective_compute("AllReduce", OP.add,
                                           replica_groups=[core_ids],
                                           ins=[z2_in[:]], outs=[z2_out[:]])
              y_f = p3a.tile([128, 12, 4], F32)
              nc.sync.dma_start(y_f[:], z2_out[:].rearrange("(c p) f -> p c f", p=128))
              for c2 in range(12):
                  nc.vector.tensor_scalar_add(y_f[:, c2, :], y_f[:, c2, :],
                                              cw2bf_t[:, c2:c2 + 1])
              nc.scalar.activation(y_f[:], y_f[:], AF.Sigmoid)
              if DEBUG:
                  nc.sync.dma_start(
                      dbg["dyf"][:].rearrange("p (a b) -> p a b", b=4), y_f[:])
              cw0A = p3a.tile([6, NCH, 128], BF16)
              cw1A = p3a.tile([6, NCH, 128], BF16)
              nc.sync.dma_start(cw0A[4:6, :, :],
                                cwaug_d[0:2, :].rearrange(
                                    "p (c m) -> p c m", m=128))
              nc.sync.dma_start(cw1A[4:6, :, :],
                                cwaug_d[2:4, :].rearrange(
                                    "p (c m) -> p c m", m=128))
              for c in range(NCH):
                  for (dstt, src) in ((cw0A, y_f[:, c, :]), (cw1A, y_f[:, 6 + c, :])):
                      pstt = sps.tile([4, 128], F32, tag="spt",
                                      name=f"ct_{c}_{dstt.name}")
                      nc.tensor.transpose(pstt[:], src, ident128[:])
                      nc.scalar.activation(dstt[0:4, c, :], pstt[:], AF.Copy,
                                           scale=0.5)

          # ============ PHASE 4: final combine ============
          with tc.tile_pool(name="p4", bufs=3) as p4, \
               tc.tile_pool(name="cwp", bufs=6, space="PSUM") as cwp:
              imgsel_r = p4.tile([6, NTOK], BF16, tag="imsr", bufs=1)
              nc.sync.dma_start(imgsel_r[0:4, :], imgsel_d[:])
              nc.sync.dma_start(imgsel_r[4:6, :], s01_all[:])
              col0 = 0
              for tl in P34_TILES:
                  cs = slice(col0, col0 + tl)
                  o1 = p4.tile([128, NCH, tl], BF16, tag="o1", name=f"o1_{col0}")
                  o2 = p4.tile([128, NCH, tl], BF16, tag="o2", name=f"o2_{col0}")
                  for c in range(NCH):
                      pc0 = cwp.tile([128, tl], F32, tag="cw", name=f"c0_{col0}_{c}")
                      nc.tensor.matmul(pc0[:], cw0A[:, c, :], imgsel_r[:, cs],
                                       start=True, stop=True)
                      pc1 = cwp.tile([128, tl], F32, tag="cw", name=f"c1_{col0}_{c}")
                      nc.tensor.matmul(pc1[:], cw1A[:, c, :], imgsel_r[:, cs],
                                       start=True, stop=True)
                      t0 = p4.tile([128, tl], BF16, tag="t0", name=f"t0_{col0}_{c}")
                      nc.vector.tensor_mul(t0[:], pc1[:], x2breg[:, c, cs])
                      nc.vector.tensor_add(o1[:, c, :], x1breg[:, c, cs], t0[:])
                      t1 = p4.tile([128, tl], BF16, tag="t1", name=f"t1_{col0}_{c}")
                      nc.vector.tensor_mul(t1[:], pc0[:], x1breg[:, c, cs])
                      nc.vector.tensor_add(o2[:, c, :], x2breg[:, c, cs], t1[:])
                  nc.sync.dma_start(r6(out1_d[:])[:, :, cs], o1[:])
                  nc.sync.dma_start(r6(out2_d[:])[:, :, cs], o2[:])
                  col0 += tl

    nc.compile()
    return nc


def _windowize(x):
    Bp = np.zeros((B, 70, 70, C), x.dtype)
    Bp[:, :64, :64, :] = x
    w = Bp.reshape(B, GRID, WIN, GRID, WIN, C).transpose(0, 1, 3, 2, 4, 5)
    w = w.reshape(NWIN_TOT, N, C)
    out = np.zeros((NCORE * NW, N, C), x.dtype)
    out[:NWIN_TOT] = w
    return out


def _unwindowize(perwin):
    w = perwin[:NWIN_TOT].reshape(B, GRID, GRID, WIN, WIN, C)
    w = w.transpose(0, 1, 3, 2, 4, 5).reshape(B, 70, 70, C)
    return np.ascontiguousarray(w[:, :64, :64, :])


def kernel(rgb_embedding, x_embedding, norm1_w, norm1_b, qkv_w, qkv_b,
           rel_pos_h, rel_pos_w, proj_w, proj_b, norm2_w, norm2_b,
           fc1_w, fc1_b, fc2_w, fc2_b, cw1_w, cw1_b, cw2_w, cw2_b,
           sw1_w, sw1_b, sw2_w, sw2_b):
    if "nc" not in _CACHE:
        _CACHE["nc"] = _build()
    nc = _CACHE["nc"]

    f32 = lambda a: np.ascontiguousarray(a, dtype=np.float32)
    bf16 = lambda a: np.ascontiguousarray(np.asarray(a, dtype=np.float32),
                                          ).astype(ml_dtypes.bfloat16)
    xw = _windowize(f32(x_embedding))
    rw = _windowize(f32(rgb_embedding))
    vm = np.zeros((NCORE * NW, N), np.float32)
    vh = np.minimum(np.maximum(64 - np.arange(GRID) * WIN, 0), WIN)
    wm = np.zeros((GRID, GRID, WIN, WIN), np.float32)
    for a in range(GRID):
        for b in range(GRID):
            wm[a, b, :vh[a], :vh[b]] = 1.0
    vm[:NWIN_TOT] = np.tile(wm.reshape(GRID * GRID, N), (B, 1))
    win_img = np.full(NCORE * NW, -1, np.int64)
    win_img[:NWIN_TOT] = np.arange(NWIN_TOT) // (GRID * GRID)

    n1w, n1b = f32(norm1_w), f32(norm1_b)
    n2w, n2b = f32(norm2_w), f32(norm2_b)
    qkw = f32(np.asarray(qkv_w)[:2 * C])
    vw = f32(np.asarray(qkv_w)[2 * C:])
    qkw_f = qkw * n1w[None, :]
    vw_f = vw * n1w[None, :]
    qkb = f32(qkv_b[:2 * C]) + qkw @ n1b
    qkb[:C] *= SCALE
    vb_f = f32(qkv_b[2 * C:]) + vw @ n1b
    projb_fold = f32(proj_b) + f32(proj_w) @ vb_f
    fc1w = f32(fc1_w)
    fc1w_f = fc1w * n2w[None, :]
    fc1b_f = f32(fc1_b) + fc1w @ n2b
    tab = np.concatenate([f32(rel_pos_h)[::-1], f32(rel_pos_w)[::-1]], axis=0)
    tab = np.ascontiguousarray(tab.T) * (1.0 / SCALE)
    cw1s = f32(cw1_w).T.copy()
    cw1s[:2 * C, :] *= 1.0 / (HH * WW)
    shared = dict(
        qkwT=bf16(qkw_f.T), qkb=qkb,
        vwT=bf16(vw_f.T),
        projwT=bf16(np.asarray(proj_w).T), projb=projb_fold,
        tab=tab,
        fc1wT=bf16(fc1w_f.T), fc1b=fc1b_f,
        fc2wT=bf16(np.asarray(fc2_w).T), fc2b=f32(fc2_b),
        sw1wT=bf16(np.asarray(sw1_w).T), sw1b=f32(sw1_b),
        sw2wT=bf16(np.asarray(sw2_w).T), sw2b=f32(sw2_b),
        cw2bf=f32(cw2_b),
        cwaug=bf16(np.array([[0.5], [0.0], [0.0], [0.5]], np.float32)
                   * np.ones((4, NCH * 128), np.float32)),
        vones=bf16(np.ones((98, NH), np.float32)),
    )
    cw2s = np.ascontiguousarray(f32(cw2_w).T)
    in_maps = []
    for c in range(NCORE):
        sl = slice(c * NW, (c + 1) * NW)
        xT = bf16(np.ascontiguousarray(xw[sl].reshape(NTOK, C).T))
        rT = bf16(np.ascontiguousarray(rw[sl].reshape(NTOK, C).T))
        mrow = vm[sl].reshape(1, NTOK).copy()
        imgm = np.zeros((4, NW), np.float32)
        imsel = np.zeros((4, NTOK), np.float32)
        for wloc in range(NW):
            im = win_img[c * NW + wloc]
            if im >= 0:
                imgm[im, wloc] = 1.0
                imsel[im, wloc * N:(wloc + 1) * N] = 1.0
        m = dict(shared)
        m.update(
            xT=xT, rgbT=rT, mask=bf16(mrow),
            negbig=bf16((mrow - 1.0) * 1e30),
            imgmask=imgm, imgneg=(imgm - 1.0) * 1e30, imgsel=bf16(imsel),
            cw1wTs=bf16(cw1s[:, c * 384:(c + 1) * 384]),
            cw1bs=f32(cw1_b[c * 384:(c + 1) * 384]).copy(),
            cw2rT=bf16(cw2s[c * 384:(c + 1) * 384, :]),
        )
        in_maps.append(m)

    trace = bool(os.environ.get("KERNEL_TRACE"))
    res = run_bass_kernel_spmd(nc, in_maps, list(range(NCORE)), trace=trace)
    if trace:
        _CACHE["exec_time_ns"] = res.exec_time_ns
    _CACHE["res"] = res
    o1 = np.zeros((NCORE * NW, N, C), np.float32)
    o2 = np.zeros((NCORE * NW, N, C), np.float32)
    for c in range(NCORE):
        sl = slice(c * NW, (c + 1) * NW)
        o1[sl] = res.results[c]["out1T"].astype(np.float32).T.reshape(NW, N, C)
        o2[sl] = res.results[c]["out2T"].astype(np.float32).T.reshape(NW, N, C)
    rgb_out = _unwindowize(o1)
    x_out = _unwindowize(o2)
    return rgb_out, x_out


# revision 31
# speedup vs baseline: 1.1105x; 1.1105x over previous
"""SAM-block (windowed attention + MLP) + FRM fusion on 8 TRN2 NeuronCores.

v2: LN affines folded into weights (host), multiplicative rel-pos bias
exp(S)*exp(gh)*exp(gw), softmax denominator via ones-column in V,
scalar_tensor_tensor fusions, rgb stats in phase 0, sw gate rows folded
into the P4 channel matmul, bf16 x input.

Set V2DEBUG=1 to add intermediate dumps (core-0 debugging).
"""
import os
import numpy as np
import ml_dtypes
import concourse.bass as bass
import concourse.bacc as bacc
import concourse.mybir as mybir
from concourse import tile
from concourse.masks import make_identity
from concourse.bass_utils import run_bass_kernel_spmd

F32 = mybir.dt.float32
BF16 = mybir.dt.bfloat16
AF = mybir.ActivationFunctionType
OP = mybir.AluOpType

B, HH, WW, C = 4, 64, 64, 768
WIN, NH, HD = 14, 12, 64
S = WIN
N = S * S
GRID = 5
NWIN_TOT = B * GRID * GRID
NCORE = 8
NW = 13
NTOK = NW * N
NT = 2 * N
NPAIR = 7
NCH = C // 128
DFF = 4 * C
SCALE = HD ** -0.5
EPS = 1e-6
P34_TILES = [392] * 6 + [196]
DEBUG = bool(os.environ.get("V2DEBUG"))

_CACHE = {}


def _build():
    nc = bacc.Bacc("TRN2", target_bir_lowering=False, debug=False)
    dt_in = {}

    def din(name, shape, dt=F32):
        dt_in[name] = nc.dram_tensor(name, shape, dt, kind="ExternalInput")
        return dt_in[name]

    xT_d = din("xT", [C, NTOK], BF16)
    rgbT_d = din("rgbT", [C, NTOK], BF16)
    mask_d = din("mask", [1, NTOK], BF16)
    negbig_d = din("negbig", [1, NTOK], BF16)
    imgmask_d = din("imgmask", [4, NW])
    imgneg_d = din("imgneg", [4, NW])
    imgsel_d = din("imgsel", [4, NTOK], BF16)
    qkwT_d = din("qkwT", [C, 2 * C], BF16)
    qkb_d = din("qkb", [2 * C])
    vwT_d = din("vwT", [C, C], BF16)
    projwT_d = din("projwT", [C, C], BF16)
    projb_d = din("projb", [C])
    tab_d = din("tab", [HD, 54])
    fc1wT_d = din("fc1wT", [C, DFF], BF16)
    fc1b_d = din("fc1b", [DFF])
    fc2wT_d = din("fc2wT", [DFF, C], BF16)
    fc2b_d = din("fc2b", [C])
    sw1wT_d = din("sw1wT", [2 * C, C], BF16)
    sw1b_d = din("sw1b", [C])
    sw2wT_d = din("sw2wT", [C, 2], BF16)
    sw2b_d = din("sw2b", [2])
    cw1wTs_d = din("cw1wTs", [4 * C, 4 * C // NCORE], BF16)
    cw1bs_d = din("cw1bs", [4 * C // NCORE])
    cw2rT_d = din("cw2rT", [4 * C // NCORE, 2 * C], BF16)
    cw2bf_d = din("cw2bf", [2 * C])
    cwaug_d = din("cwaug", [4, NCH * 128], BF16)
    vones_d = din("vones", [98, NH], BF16)
    out1_d = nc.dram_tensor("out1T", [C, NTOK], BF16, kind="ExternalOutput")
    out2_d = nc.dram_tensor("out2T", [C, NTOK], BF16, kind="ExternalOutput")
    if DEBUG:
        dbg = {
            "dlnx": nc.dram_tensor("dlnx", [128, NCH * NT], BF16, kind="ExternalOutput"),
            "deh": nc.dram_tensor("deh", [98, NH * 14], BF16, kind="ExternalOutput"),
            "dew": nc.dram_tensor("dew", [98, NH * 14], BF16, kind="ExternalOutput"),
            "dP": nc.dram_tensor("dP", [98, 2 * 196], BF16, kind="ExternalOutput"),
            "dz": nc.dram_tensor("dz", [1, 2 * 196], F32, kind="ExternalOutput"),
            "dzbb": nc.dram_tensor("dzbb", [64, 2 * 196], F32, kind="ExternalOutput"),
            "dattn": nc.dram_tensor("dattn", [128, NCH * NT], BF16, kind="ExternalOutput"),
            "dx2a": nc.dram_tensor("dx2a", [C, NTOK], BF16, kind="ExternalOutput"),
            "dx2b": nc.dram_tensor("dx2b", [128, NCH * NT], BF16, kind="ExternalOutput"),
            "ds01": nc.dram_tensor("ds01", [2, NTOK], BF16, kind="ExternalOutput"),
            "dyf": nc.dram_tensor("dyf", [128, 12 * 4], F32, kind="ExternalOutput"),
        }

    core_ids = list(range(NCORE))
    r6 = lambda ap: ap.rearrange("(c p) n -> p c n", p=128)
    rcol = lambda ap: ap.rearrange("(c p) -> p c", p=128)

    with tile.TileContext(nc) as tc:
      with tc.tile_pool(name="dram", bufs=1, space="DRAM") as dramp, \
           tc.tile_pool(name="fbp", bufs=6, space="DRAM") as fbp, \
           tc.tile_pool(name="cst", bufs=1) as cp, \
           tc.tile_pool(name="pers", bufs=1) as pers, \
           tc.tile_pool(name="stg", bufs=2) as stg:
        x2a_d = dramp.tile([C, NTOK], BF16)
        csum_in = dramp.tile([128, 48], F32)
        csum_out = dramp.tile([128, 48], F32, addr_space="Shared")
        cmax_in = dramp.tile([128, 48], F32)
        cmax_out = dramp.tile([128, 48], F32, addr_space="Shared")
        z2_in = dramp.tile([2 * C, 4], F32)
        z2_out = dramp.tile([2 * C, 4], F32, addr_space="Shared")

        def load_rows(src, n=C):
            t = cp.tile([128, n // 128], F32, name="rows_" + src.tensor.name)
            nc.sync.dma_start(t[:], rcol(src))
            return t

        qkb_t = load_rows(qkb_d[:], 2 * C)
        projb_t = load_rows(projb_d[:])
        fc1b_t = load_rows(fc1b_d[:], DFF)
        fc2b_t = load_rows(fc2b_d[:])
        sw1b_t = load_rows(sw1b_d[:])
        sw2b_t = cp.tile([2, 1], F32)
        nc.sync.dma_start(sw2b_t[:, 0], sw2b_d[:])
        cw1bs_t = load_rows(cw1bs_d[:], 4 * C // NCORE)
        cw2bf_t = load_rows(cw2bf_d[:], 2 * C)
        tabf = stg.tile([128, 2, 54], F32, tag="st1")
        nc.any.memset(tabf[:], 0.0)
        nc.sync.dma_start(tabf[0:64, 0, :], tab_d[:])
        nc.sync.dma_start(tabf[64:128, 1, :], tab_d[:])
        ones_f = stg.tile([128, 1], F32, tag="st1")
        nc.any.memset(ones_f[:], 1.0)
        ones_b = cp.tile([128, 1], BF16)
        nc.vector.tensor_copy(ones_b[:], ones_f[:])

        def load_bf(pool_, shape3, src_ap, nm):
            r = pool_.tile(shape3, BF16, name="w_" + nm)
            nc.sync.dma_start(r[:], src_ap)
            return r

        # ========== PHASE 0: rgb FRM window stats (overlaps attention) ======
        negrow = stg.tile([1, NTOK], BF16, tag="rgb0", name="negrow")
        nc.sync.dma_start(negrow[:], negbig_d[:])
        neg_full = pers.tile([128, NTOK], BF16)
        nc.gpsimd.partition_broadcast(neg_full[:], negrow[:])
        mskrow = stg.tile([1, NTOK], BF16, tag="rgb0", name="mskrow")
        nc.sync.dma_start(mskrow[:], mask_d[:])
        msk_full = pers.tile([128, NTOK], BF16)
        nc.gpsimd.partition_broadcast(msk_full[:], mskrow[:])
        W_s1 = pers.tile([128, NCH, NW, 1], F32)
        W_m1 = pers.tile([128, NCH, NW, 1], F32)
        col0 = 0
        for ti, tl in enumerate(P34_TILES):
            nwt = tl // N
            cs = slice(col0, col0 + tl)
            rgb0 = stg.tile([128, NCH, tl], BF16, tag="rgb0", name=f"rgb0_{ti}")
            nc.sync.dma_start(rgb0[:], r6(rgbT_d[:])[:, :, cs])
            for c in range(NCH):
                x1n = stg.tile([128, tl], BF16, tag="x1n", name=f"x1n_{ti}_{c}")
                nc.vector.tensor_add(x1n[:], rgb0[:, c, :], neg_full[:, cs])
                nc.vector.tensor_reduce(
                    W_s1[:, c, 2 * ti:2 * ti + nwt, :],
                    rgb0[:, c, :].rearrange("p (w n) -> p w n", n=N),
                    mybir.AxisListType.X, OP.add)
                nc.vector.tensor_reduce(
                    W_m1[:, c, 2 * ti:2 * ti + nwt, :],
                    x1n[:].rearrange("p (w n) -> p w n", n=N),
                    mybir.AxisListType.X, OP.max)
            col0 += tl

        # ==================== PHASE 1: attention ====================
        with tc.tile_pool(name="w1", bufs=1) as wp1, \
             tc.tile_pool(name="p1", bufs=2) as p1, \
             tc.tile_pool(name="p1s", bufs=2) as p1s, \
             tc.tile_pool(name="p1p", bufs=10) as p1p, \
             tc.tile_pool(name="p1f", bufs=2) as p1f, \
             tc.tile_pool(name="gen", bufs=2, space="PSUM") as psg, \
             tc.tile_pool(name="scp", bufs=2, space="PSUM") as pscp, \
             tc.tile_pool(name="psa", bufs=2, space="PSUM") as psa, \
             tc.tile_pool(name="pst", bufs=2, space="PSUM") as pst:
            qkwT = load_bf(wp1, [128, NCH, 2 * C], r6(qkwT_d[:]), "qk")
            vwT = load_bf(wp1, [128, NCH, C], r6(vwT_d[:]), "v")
            projwT = load_bf(wp1, [128, NCH, C], r6(projwT_d[:]), "pj")
            tab2b = wp1.tile([128, 2, 54], BF16)
            nc.vector.tensor_copy(tab2b[:], tabf[:])
            idstage = wp1.tile([128, 128], F32, tag="idst", bufs=1)
            make_identity(nc, idstage)
            ident98b = wp1.tile([98, 98], BF16)
            nc.vector.tensor_copy(ident98b[:], idstage[0:98, 0:98])

            kz0 = p1.tile([128, NH, NT], BF16, tag="kz0", bufs=1)
            zst = wp1.tile([64, NT], BF16, tag="wstage", bufs=1)
            nc.any.memset(zst[:], 0.0)
            for j in range(NH // 2):
                nc.vector.tensor_copy(kz0[64:128, 2 * j, :], zst[:])
                nc.vector.tensor_copy(kz0[0:64, 2 * j + 1, :], zst[:])

            for blk in range(NPAIR):
                degen = blk == NPAIR - 1
                wr = [0] if degen else [0, 1]
                c0 = blk * NT
                cols = slice(c0, c0 + NT) if not degen else None
                xt = p1.tile([128, NCH, NT], BF16, tag="xt")
                if degen:
                    nc.sync.dma_start(xt[:, :, 0:N], r6(xT_d[:])[:, :, 2352:2548])
                    nc.sync.dma_start(xt[:, :, N:NT], r6(xT_d[:])[:, :, 2352:2548])
                else:
                    nc.sync.dma_start(xt[:], r6(xT_d[:])[:, :, cols])

                s1 = psg.tile([128, 512], F32, tag="gen", name=f"s1_{blk}")
                s2 = psg.tile([128, 512], F32, tag="gen", name=f"s2_{blk}")
                for c in range(NCH):
                    nc.tensor.matmul(s1[0:1, 0:NT], ones_b[:], xt[:, c, :],
                                     start=(c == 0), stop=(c == NCH - 1))
                for c in range(NCH):
                    xsqc = p1s.tile([128, NT], BF16, tag="xsqc",
                                    name=f"xsq_{blk}_{c}")
                    nc.vector.tensor_mul(xsqc[:], xt[:, c, :], xt[:, c, :])
                    nc.tensor.matmul(s2[0:1, 0:NT], ones_b[:], xsqc[:],
                                     start=(c == 0), stop=(c == NCH - 1))
                rws = p1f.tile([1, 3, NT], F32, tag="rws", name=f"rw1_{blk}")
                nc.vector.tensor_scalar_mul(rws[:, 0, :], s1[0:1, 0:NT], 1.0 / C)
                nc.vector.tensor_mul(rws[:, 1, :], rws[:, 0, :], rws[:, 0, :])
                nc.vector.tensor_scalar_mul(rws[:, 2, :], s2[0:1, 0:NT], 1.0 / C)
                nc.vector.tensor_sub(rws[:, 1, :], rws[:, 2, :], rws[:, 1, :])
                nc.vector.tensor_scalar_add(rws[:, 1, :], rws[:, 1, :], EPS)
                nc.scalar.activation(rws[:, 2, :], rws[:, 1, :], AF.Sqrt)
                nc.vector.reciprocal(rws[:, 1, :], rws[:, 2, :])
                rwb = p1f.tile([1, 2, NT], BF16, tag="rwb", name=f"rb1_{blk}")
                nc.vector.tensor_copy(rwb[:, 0, :], rws[:, 0, :])
                nc.vector.tensor_copy(rwb[:, 1, :], rws[:, 1, :])
                m_b = p1f.tile([128, NT], BF16, tag="m_b")
                nc.gpsimd.partition_broadcast(m_b[:], rwb[:, 0, :])
                inv_b = p1f.tile([128, NT], BF16, tag="inv_b")
                nc.gpsimd.partition_broadcast(inv_b[:], rwb[:, 1, :])
                lnx_r = p1.tile([128, NCH, NT], BF16, tag="lnx_r")
                for c in range(NCH):
                    u = p1s.tile([128, NT], BF16, tag="u", name=f"u_{blk}_{c}")
                    nc.vector.tensor_sub(u[:], xt[:, c, :], m_b[:])
                    nc.vector.tensor_mul(lnx_r[:, c, :], u[:], inv_b[:])
                if DEBUG and blk == 0:
                    nc.sync.dma_start(
                        dbg["dlnx"][:].rearrange("p (c n) -> p c n", n=NT),
                        lnx_r[:])

                qT = p1.tile([128, NCH, NT], BF16, tag="qT")
                kzt = kz0
                for mc in range(12):
                    ps = psg.tile([128, 512], F32, tag="gen", name=f"qk_{blk}_{mc}")
                    for kc in range(NCH):
                        nc.tensor.matmul(ps[0:128, 0:NT],
                                         qkwT[:, kc, mc * 128:(mc + 1) * 128],
                                         lnx_r[:, kc, :], start=(kc == 0),
                                         stop=(kc == NCH - 1))
                    if mc < NCH:
                        nc.vector.tensor_scalar(qT[:, mc, :], ps[0:128, 0:NT],
                                                SCALE, qkb_t[:, mc:mc + 1],
                                                OP.mult, OP.add)
                    else:
                        j = mc - NCH
                        nc.vector.tensor_scalar_add(kzt[0:64, 2 * j, :],
                                                    ps[0:64, 0:NT],
                                                    qkb_t[0:64, mc:mc + 1])
                        nc.vector.tensor_scalar_add(kzt[64:128, 2 * j + 1, :],
                                                    ps[64:128, 0:NT],
                                                    qkb_t[64:128, mc:mc + 1])

                v_sb = {}
                for w in wr:
                    for tch in range(2):
                        t0 = w * N + tch * 98
                        vt = p1.tile([98, NH, 65], BF16, tag=f"v{w}{tch}",
                                     name=f"v{w}{tch}_{blk}")
                        ps_a = psg.tile([128, 512], F32, tag="gen",
                                        name=f"va_{blk}_{w}_{tch}")
                        ps_b = psg.tile([128, 512], F32, tag="gen",
                                        name=f"vb_{blk}_{w}_{tch}")
                        for kc in range(NCH):
                            nc.tensor.matmul(ps_a[0:98, 0:512],
                                             lnx_r[:, kc, t0:t0 + 98],
                                             vwT[:, kc, 0:512], start=(kc == 0),
                                             stop=(kc == NCH - 1))
                        for kc in range(NCH):
                            nc.tensor.matmul(ps_b[0:98, 0:256],
                                             lnx_r[:, kc, t0:t0 + 98],
                                             vwT[:, kc, 512:768], start=(kc == 0),
                                             stop=(kc == NCH - 1))
                        nc.scalar.copy(vt[:, 0:8, 0:64],
                                       ps_a[0:98, 0:512].rearrange(
                                           "p (h d) -> p h d", d=64))
                        nc.scalar.copy(vt[:, 8:12, 0:64],
                                       ps_b[0:98, 0:256].rearrange(
                                           "p (h d) -> p h d", d=64))
                        nc.sync.dma_start(vt[:, :, 64], vones_d[:])
                        v_sb[(w, tch)] = vt

                e_h, e_w = {}, {}
                for w in wr:
                    gh2 = p1f.tile([98, NH, 2, 14, 1], BF16, tag=f"gh{w}",
                                   name=f"gh{w}_{blk}")
                    gw2 = p1f.tile([98, NH, 2, 1, 14], BF16, tag=f"gw{w}",
                                   name=f"gw{w}_{blk}")
                    for qc in range(2):
                        ps1 = psg.tile([128, 512], F32, tag="gen",
                                       name=f"f1_{blk}_{w}_{qc}")
                        ps2 = psg.tile([128, 512], F32, tag="gen",
                                       name=f"f2_{blk}_{w}_{qc}")
                        for h in range(NH):
                            dst = (ps1[0:98, h * 54:h * 54 + 54] if h < 9 else
                                   ps2[0:98, (h - 9) * 54:(h - 9) * 54 + 54])
                            lhsT = qT[:, h // 2, w * N + qc * 98: w * N + qc * 98 + 98]
                            nc.tensor.matmul(dst, lhsT, tab2b[:, h % 2, :],
                                             start=True, stop=True)
                        f = p1s.tile([98, NH, 54], BF16, tag="ftile",
                                     name=f"f{w}{qc}_{blk}")
                        nc.scalar.copy(f[:, 0:9, :],
                                       ps1[0:98, 0:486].rearrange(
                                           "p (h t) -> p h t", t=54))
                        nc.scalar.copy(f[:, 9:12, :],
                                       ps2[0:98, 0:162].rearrange(
                                           "p (h t) -> p h t", t=54))
                        fd = fbp.tile([98, NH, 54], BF16, tag="fb",
                                      name=f"fd{w}{qc}_{blk}")
                        nc.scalar.dma_start(fd[:], f[:])
                        for hb in range(7):
                            h_glob = qc * 7 + hb
                            nc.sync.dma_start(
                                gh2[hb * 14:(hb + 1) * 14, :, qc, :, 0],
                                f[hb * 14:(hb + 1) * 14, :, 13 - h_glob:27 - h_glob])
                            src = bass.AP(fd[:].tensor, hb * 14 * 648 + 40,
                                          [[647, 14], [54, NH], [1, 14]])
                            nc.scalar.dma_start(
                                gw2[hb * 14:(hb + 1) * 14, :, qc, 0, :], src)
                    eh2 = p1f.tile([98, NH, 2, 14, 1], BF16, tag=f"eh{w}",
                                   name=f"eh{w}_{blk}")
                    ew2 = p1f.tile([98, NH, 2, 1, 14], BF16, tag=f"ew{w}",
                                   name=f"ew{w}_{blk}")
                    nc.scalar.activation(eh2[:], gh2[:], AF.Exp)
                    nc.scalar.activation(ew2[:], gw2[:], AF.Exp)
                    e_h[w], e_w[w] = eh2, ew2
                    if DEBUG and blk == 0 and w == 0:
                        nc.sync.dma_start(
                            dbg["deh"][:].rearrange("p (a b) -> p a b", b=14),
                            eh2[:, :, 0, :, 0])
                        nc.sync.dma_start(
                            dbg["dew"][:].rearrange("p (a b) -> p a b", b=14),
                            ew2[:, :, 0, 0, :])

                attn_outT = p1.tile([128, NCH, NT], BF16, tag="attn_outT")
                for h in range(NH):
                    psA_w = {}
                    zrow = p1f.tile([1, 2, 196], F32, tag="zrow",
                                    name=f"zr_{blk}_{h}")
                    zsb = p1f.tile([1, 2, 196], F32, tag="zsb",
                                   name=f"zsb_{blk}_{h}")
                    for w in wr:
                        P = p1p.tile([98, 2, 196], BF16, tag="P",
                                     name=f"P_{blk}_{h}_{w}")
                        sps2 = pscp.tile([98, 2, 196], F32, tag="sc",
                                         name=f"sc_{blk}_{h}_{w}")
                        for qc in range(2):
                            lhsT = qT[:, h // 2,
                                      w * N + qc * 98: w * N + qc * 98 + 98]
                            nc.tensor.matmul(sps2[:, qc, :], lhsT,
                                             kzt[:, h, w * N:w * N + 196],
                                             start=True, stop=True)
                        nc.scalar.activation(P[:], sps2[:], AF.Exp)
                        P4v = P[:].rearrange("p a (b c) -> p a b c", c=14)
                        nc.vector.tensor_mul(
                            P4v, P4v,
                            e_h[w][:, h, :, :, :].broadcast_to([98, 2, 14, 14]))
                        nc.vector.tensor_mul(
                            P4v, P4v,
                            e_w[w][:, h, :, :, :].broadcast_to([98, 2, 14, 14]))
                        if DEBUG and blk == 0 and h == 0 and w == 0:
                            nc.sync.dma_start(
                                dbg["dP"][:].rearrange("p (a b) -> p a b", b=196),
                                P[:])
                        PT_sb = []
                        for kc in range(2):
                            ptp = pst.tile([98, 2, 98], BF16, tag="pt",
                                           name=f"pt_{blk}_{h}_{w}_{kc}")
                            for qc in range(2):
                                nc.tensor.transpose(
                                    ptp[:, qc, :],
                                    P[:, qc, kc * 98:kc * 98 + 98], ident98b[:])
                            sb = p1p.tile([98, 2, 98], BF16, tag="PTsb",
                                          name=f"ptsb_{blk}_{h}_{w}_{kc}")
                            nc.scalar.copy(sb[:], ptp[:])
                            PT_sb.append(sb)
                        psA = psa.tile([65, 196], F32, tag="psAV",
                                       name=f"av_{blk}_{h}_{w}")
                        for kc in range(2):
                            nc.tensor.matmul(
                                psA[:],
                                v_sb[(w, kc)][:, h, :],
                                PT_sb[kc][:].rearrange("p a b -> p (a b)"),
                                start=(kc == 0), stop=(kc == 1))
                        # stage z to partition-0 SBUF: the custom-DVE
                        # reciprocal misreads PSUM at partition offset 64
                        nc.scalar.copy(zsb[:, w, :], psA[64:65, :])
                        psA_w[w] = psA
                    zbb = p1f.tile([64, 2, 196], F32, tag="zbb",
                                   name=f"zbb_{blk}_{h}")
                    if degen:
                        nc.vector.reciprocal_approx_fast(zrow[:, 0, :],
                                                         zsb[:, 0, :])
                        nc.gpsimd.partition_broadcast(zbb[:, 0, :], zrow[:, 0, :])
                    else:
                        nc.vector.reciprocal_approx_fast(
                            zrow[:].rearrange("p a b -> p (a b)"),
                            zsb[:].rearrange("p a b -> p (a b)"))
                        nc.gpsimd.partition_broadcast(
                            zbb[:].rearrange("p a b -> p (a b)"),
                            zrow[:].rearrange("p a b -> p (a b)"))
                    if DEBUG and blk == 0 and h == 0:
                        nc.sync.dma_start(
                            dbg["dz"][:].rearrange("p (a b) -> p a b", b=196),
                            zrow[:])
                        nc.sync.dma_start(
                            dbg["dzbb"][:].rearrange("p (a b) -> p a b", b=196),
                            zbb[:])
                    for w in wr:
                        nc.vector.tensor_mul(
                            attn_outT[(h % 2) * 64:(h % 2) * 64 + 64, h // 2,
                                      w * N:w * N + 196],
                            psA_w[w][0:64, :], zbb[:, w, :])
                if DEBUG and blk == 0:
                    nc.sync.dma_start(
                        dbg["dattn"][:].rearrange("p (c n) -> p c n", n=NT),
                        attn_outT[:])

                nout = N if degen else NT
                outc = p1s.tile([128, NCH, NT], BF16, tag="outc")
                for mc in range(NCH):
                    ps = psg.tile([128, 512], F32, tag="gen", name=f"pj_{blk}_{mc}")
                    for kc in range(NCH):
                        nc.tensor.matmul(ps[0:128, 0:nout],
                                         projwT[:, kc, mc * 128:(mc + 1) * 128],
                                         attn_outT[:, kc, 0:nout], start=(kc == 0),
                                         stop=(kc == NCH - 1))
                    nc.vector.scalar_tensor_tensor(
                        outc[:, mc, 0:nout], ps[0:128, 0:nout],
                        projb_t[:, mc:mc + 1], xt[:, mc, 0:nout],
                        OP.add, OP.add)
                dstc = (r6(x2a_d[:])[:, :, 2352:2548] if degen
                        else r6(x2a_d[:])[:, :, cols])
                nc.sync.dma_start(dstc, outc[:, :, 0:nout])

        if DEBUG:
            nc.sync.dma_start(dbg["dx2a"][:], x2a_d[:])

        # ====== PHASE 2: MLP + FRM stats ======
        with tc.tile_pool(name="p3a", bufs=1) as p3a:
          x2breg = p3a.tile([128, NCH, NTOK], BF16, name="x2breg")
          W_s2 = p3a.tile([128, NCH, NW, 1], F32)
          W_m2 = p3a.tile([128, NCH, NW, 1], F32)
          s01_all = p3a.tile([2, NTOK], BF16)

          with tc.tile_pool(name="w2", bufs=1) as wp2, \
               tc.tile_pool(name="p2", bufs=2) as p2, \
               tc.tile_pool(name="p2f", bufs=2) as p2f, \
               tc.tile_pool(name="ps1p", bufs=2, space="PSUM") as ps1p, \
               tc.tile_pool(name="ps2p", bufs=6, space="PSUM") as ps2p:
            fc1wT = load_bf(wp2, [128, NCH, DFF], r6(fc1wT_d[:]), "fc1")
            fc2wT = load_bf(wp2, [128, DFF // 128, C], r6(fc2wT_d[:]), "fc2")
            col0 = 0
            for ti, tl in enumerate(P34_TILES):
                nwt = tl // N
                cs = slice(col0, col0 + tl)
                xa = p2.tile([128, NCH, tl], BF16, tag="xa", name=f"xa_{col0}")
                nc.sync.dma_start(xa[:], r6(x2a_d[:])[:, :, cs])
                s1 = ps1p.tile([128, tl], F32, tag="ps1", name=f"s1_{col0}")
                s2 = ps1p.tile([128, tl], F32, tag="ps1", name=f"s2_{col0}")
                for c in range(NCH):
                    nc.tensor.matmul(s1[0:1, :], ones_b[:], xa[:, c, :],
                                     start=(c == 0), stop=(c == NCH - 1))
                for c in range(NCH):
                    xsqc = p2.tile([128, tl], BF16, tag="xsqc",
                                   name=f"xsq2_{col0}_{c}")
                    nc.vector.tensor_mul(xsqc[:], xa[:, c, :], xa[:, c, :])
                    nc.tensor.matmul(s2[0:1, :], ones_b[:], xsqc[:],
                                     start=(c == 0), stop=(c == NCH - 1))
                rws = p2f.tile([1, 3, tl], F32, tag="rws", name=f"rw_{col0}",
                               bufs=1)
                nc.vector.tensor_scalar_mul(rws[:, 0, :], s1[0:1, :], 1.0 / C)
                nc.vector.tensor_mul(rws[:, 1, :], rws[:, 0, :], rws[:, 0, :])
                nc.vector.tensor_scalar_mul(rws[:, 2, :], s2[0:1, :], 1.0 / C)
                nc.vector.tensor_sub(rws[:, 1, :], rws[:, 2, :], rws[:, 1, :])
                nc.vector.tensor_scalar_add(rws[:, 1, :], rws[:, 1, :], EPS)
                nc.scalar.activation(rws[:, 2, :], rws[:, 1, :], AF.Sqrt)
                nc.vector.reciprocal(rws[:, 1, :], rws[:, 2, :])
                rwb = p2f.tile([1, 2, tl], BF16, tag="rwb", name=f"rb_{col0}")
                nc.vector.tensor_copy(rwb[:, 0, :], rws[:, 0, :])
                nc.vector.tensor_copy(rwb[:, 1, :], rws[:, 1, :])
                m_b = p2f.tile([128, tl], BF16, tag="m_b", name=f"mb_{col0}")
                nc.gpsimd.partition_broadcast(m_b[:], rwb[:, 0, :])
                inv_b = p2f.tile([128, tl], BF16, tag="inv_b", name=f"ib_{col0}")
                nc.gpsimd.partition_broadcast(inv_b[:], rwb[:, 1, :])
                ln2_r = p2.tile([128, NCH, tl], BF16, tag="ln2_r", name=f"l2_{col0}")
                for c in range(NCH):
                    u = p2.tile([128, tl], BF16, tag="u", name=f"u_{col0}_{c}")
                    nc.vector.tensor_sub(u[:], xa[:, c, :], m_b[:])
                    nc.vector.tensor_mul(ln2_r[:, c, :], u[:], inv_b[:])
                psum2 = [ps2p.tile([128, tl], F32, tag="ps2", name=f"ps2_{col0}_{m}")
                         for m in range(NCH)]
                for kc in range(DFF // 128):
                    ps1 = ps1p.tile([128, tl], F32, tag="ps1", name=f"ps1_{col0}_{kc}")
                    for c in range(NCH):
                        nc.tensor.matmul(ps1[:], fc1wT[:, c, kc * 128:(kc + 1) * 128],
                                         ln2_r[:, c, :], start=(c == 0),
                                         stop=(c == NCH - 1))
                    h1 = p2.tile([128, tl], BF16, tag="h1", name=f"h1_{col0}_{kc}")
                    nc.scalar.activation(h1[:], ps1[:], AF.Gelu,
                                         bias=fc1b_t[:, kc:kc + 1], scale=1.0)
                    for mc in range(NCH):
                        nc.tensor.matmul(psum2[mc][:],
                                         fc2wT[:, kc, mc * 128:(mc + 1) * 128],
                                         h1[:], start=(kc == 0),
                                         stop=(kc == DFF // 128 - 1))
                for mc in range(NCH):
                    x2f = p2.tile([128, tl], BF16, tag="x2f", name=f"x2f_{col0}_{mc}")
                    nc.vector.scalar_tensor_tensor(
                        x2f[:], psum2[mc][:], fc2b_t[:, mc:mc + 1],
                        xa[:, mc, :], OP.add, OP.add)
                    nc.vector.tensor_mul(x2breg[:, mc, cs], x2f[:],
                                         msk_full[:, cs])
                    xmn = p2.tile([128, tl], BF16, tag="xmn", name=f"xn_{col0}_{mc}")
                    nc.vector.tensor_add(xmn[:], x2breg[:, mc, cs],
                                         neg_full[:, cs])
                    nc.vector.tensor_reduce(
                        W_s2[:, mc, 2 * ti:2 * ti + nwt, :],
                        x2breg[:, mc, cs].rearrange("p (w n) -> p w n", n=N),
                        mybir.AxisListType.X, OP.add)
                    nc.vector.tensor_reduce(
                        W_m2[:, mc, 2 * ti:2 * ti + nwt, :],
                        xmn[:].rearrange("p (w n) -> p w n", n=N),
                        mybir.AxisListType.X, OP.max)
                col0 += tl
          if DEBUG:
              nc.sync.dma_start(
                  dbg["dx2b"][:].rearrange("p (c n) -> p c n", n=NT),
                  x2breg[:, :, 0:NT])

          # rgb resident for sw path + final combine (space freed by wp2)
          x1breg = p3a.tile([128, NCH, NTOK], BF16, name="x1breg")
          nc.sync.dma_start(x1breg[:], r6(rgbT_d[:]))

          # ====== image stats combine + collectives + channel MLP ======
          with tc.tile_pool(name="p3", bufs=2) as p3, \
               tc.tile_pool(name="w3", bufs=1) as wp3, \
               tc.tile_pool(name="sps", bufs=1, space="PSUM") as sps:
              ident128 = p3a.tile([128, 128], F32)
              idstage = p3.tile([128, 128], F32, tag="idst")
              make_identity(nc, idstage)
              nc.vector.tensor_copy(ident128[:], idstage[:])
              imb = p3a.tile([128, 4, NW], F32)
              inb = p3a.tile([128, 4, NW], F32)
              for i in range(4):
                  r = stg.tile([1, NW], F32, tag="imrow")
                  nc.sync.dma_start(r[:], imgmask_d[i:i + 1, :])
                  nc.gpsimd.partition_broadcast(imb[:, i, :], r[:])
                  r2 = stg.tile([1, NW], F32, tag="imrow")
                  nc.sync.dma_start(r2[:], imgneg_d[i:i + 1, :])
                  nc.gpsimd.partition_broadcast(inb[:, i, :], r2[:])
              stat_s = p3a.tile([128, NCH, 2, 4], F32)
              stat_m = p3a.tile([128, NCH, 2, 4], F32)
              for k, Wt in ((0, W_s1), (1, W_s2)):
                  for i in range(4):
                      t = p3.tile([128, NCH, NW], F32, tag="cmb", name=f"cmb_{k}_{i}")
                      nc.vector.tensor_mul(t[:], Wt[:, :, :, 0],
                                           imb[:, i, None, :].broadcast_to(
                                               [128, NCH, NW]))
                      nc.vector.tensor_reduce(stat_s[:, :, k, i:i + 1], t[:],
                                              mybir.AxisListType.X, OP.add)
              for k, Wt in ((0, W_m1), (1, W_m2)):
                  for i in range(4):
                      t = p3.tile([128, NCH, NW], F32, tag="cmb", name=f"cmbm_{k}_{i}")
                      nc.vector.tensor_mul(t[:], Wt[:, :, :, 0],
                                           imb[:, i, None, :].broadcast_to(
                                               [128, NCH, NW]))
                      nc.vector.tensor_add(t[:], t[:],
                                           inb[:, i, None, :].broadcast_to(
                                               [128, NCH, NW]))
                      nc.vector.tensor_reduce(stat_m[:, :, k, i:i + 1], t[:],
                                              mybir.AxisListType.X, OP.max)
              nc.sync.dma_start(csum_in[:], stat_s[:].rearrange("p a b c -> p (a b c)"))
              nc.sync.dma_start(cmax_in[:], stat_m[:].rearrange("p a b c -> p (a b c)"))
              nc.gpsimd.collective_compute("AllReduce", OP.add,
                                           replica_groups=[core_ids],
                                           ins=[csum_in[:]], outs=[csum_out[:]])
              nc.gpsimd.collective_compute("AllReduce", OP.max,
                                           replica_groups=[core_ids],
                                           ins=[cmax_in[:]], outs=[cmax_out[:]])

              sw1wT = load_bf(wp3, [128, 2 * NCH, C], sw1wT_d[:].rearrange(
                  "(c p) m -> p c m", p=128), "sw1")
              sw2wT = load_bf(wp3, [128, NCH, 2], r6(sw2wT_d[:]), "sw2")
              with tc.tile_pool(name="p3b", bufs=2) as p3b, \
                   tc.tile_pool(name="zps", bufs=6, space="PSUM") as zps:
                col0 = 0
                for ti, tl in enumerate(P34_TILES):
                    cs = slice(col0, col0 + tl)
                    zpsl = [zps.tile([128, tl], F32, tag="zp", name=f"zp_{col0}_{m}")
                            for m in range(NCH)]
                    for mc in range(NCH):
                        for kc in range(2 * NCH):
                            rhs = (x1breg[:, kc, cs] if kc < NCH
                                   else x2breg[:, kc - NCH, cs])
                            nc.tensor.matmul(zpsl[mc][:],
                                             sw1wT[:, kc, mc * 128:(mc + 1) * 128],
                                             rhs, start=(kc == 0),
                                             stop=(kc == 2 * NCH - 1))
                    z_r = p3b.tile([128, NCH, tl], BF16, tag="z_r", name=f"zr_{col0}")
                    for mc in range(NCH):
                        nc.vector.tensor_scalar(z_r[:, mc, :], zpsl[mc][:],
                                                sw1b_t[:, mc:mc + 1], 0.0,
                                                OP.add, OP.max)
                    sps_t = sps.tile([2, tl], F32, tag="sp", name=f"sp_{col0}")
                    for kc in range(NCH):
                        nc.tensor.matmul(sps_t[:], sw2wT[:, kc, :], z_r[:, kc, :],
                                         start=(kc == 0), stop=(kc == NCH - 1))
                    nc.vector.tensor_scalar_add(s01_all[:, cs], sps_t[:], sw2b_t[:])
                    col0 += tl
                nc.scalar.activation(s01_all[:], s01_all[:], AF.Sigmoid)
                if DEBUG:
                    nc.sync.dma_start(dbg["ds01"][:], s01_all[:])

              ycat_f = p3a.tile([128, 24, 4], F32)
              cso4 = csum_out[:].rearrange("p (c k i) -> p c k i", k=2, i=4)
              cmo4 = cmax_out[:].rearrange("p (c k i) -> p c k i", k=2, i=4)
              nc.sync.dma_start(ycat_f[:, 0:6, :], cso4[:, :, 0, :])
              nc.sync.dma_start(ycat_f[:, 6:12, :], cso4[:, :, 1, :])
              nc.sync.dma_start(ycat_f[:, 12:18, :], cmo4[:, :, 0, :])
              nc.sync.dma_start(ycat_f[:, 18:24, :], cmo4[:, :, 1, :])
              ycat_r = p3a.tile([128, 24, 4], BF16)
              nc.vector.tensor_copy(ycat_r[:], ycat_f[:])
              cw1wTs = load_bf(wp3, [128, 24, 4 * C // NCORE],
                               cw1wTs_d[:].rearrange("(c p) m -> p c m", p=128), "cw1")
              z1sb = p3a.tile([128, 3, 4], F32)
              for mc in range(3):
                  ps = sps.tile([128, 4], F32, tag="sp", name=f"z1ps_{mc}")
                  for kc in range(24):
                      nc.tensor.matmul(ps[:], cw1wTs[:, kc, mc * 128:(mc + 1) * 128],
                                       ycat_r[:, kc, :], start=(kc == 0), stop=(kc == 23))
                  nc.scalar.activation(z1sb[:, mc, :], ps[:], AF.Relu,
                                       bias=cw1bs_t[:, mc:mc + 1], scale=1.0)
              z1r = p3a.tile([128, 3, 4], BF16)
              nc.vector.tensor_copy(z1r[:], z1sb[:])
              cw2r = load_bf(wp3, [128, 3, 2 * C],
                             cw2rT_d[:].rearrange("(c p) m -> p c m", p=128),
                             "cw2")
              z2p = p3a.tile([128, 12, 4], F32)
              for mc in range(12):
                  ps = sps.tile([128, 4], F32, tag="sp", name=f"z2ps_{mc}")
                  for kc in range(3):
                      nc.tensor.matmul(ps[:], cw2r[:, kc, mc * 128:(mc + 1) * 128],
                                       z1r[:, kc, :], start=(kc == 0),
                                       stop=(kc == 2))
                  nc.scalar.copy(z2p[:, mc, :], ps[:])
              nc.sync.dma_start(z2_in[:].rearrange("(m p) f -> p m f", p=128),
                                z2p[:])
              nc.gpsimd.collective_compute("AllReduce", OP.add,
                                           replica_groups=[core_ids],
                                           ins=[z2_in[:]], outs=[z2_out[:]])
              y_f = p3a.tile([128, 12, 4], F32)
              nc.sync.dma_start(y_f[:], z2_out[:].rearrange("(c p) f -> p c f", p=128))
              for c2 in range(12):
                  nc.vector.tensor_scalar_add(y_f[:, c2, :], y_f[:, c2, :],
                                              cw2bf_t[:, c2:c2 + 1])
              nc.scalar.activation(y_f[:], y_f[:], AF.Sigmoid)
              if DEBUG:
                  nc.sync.dma_start(
                      dbg["dyf"][:].rearrange("p (a b) -> p a b", b=4), y_f[:])
              cw0A = p3a.tile([6, NCH, 128], BF16)
              cw1A = p3a.tile([6, NCH, 128], BF16)
              nc.sync.dma_start(cw0A[4:6, :, :],
                                cwaug_d[0:2, :].rearrange(
                                    "p (c m) -> p c m", m=128))
              nc.sync.dma_start(cw1A[4:6, :, :],
                                cwaug_d[2:4, :].rearrange(
                                    "p (c m) -> p c m", m=128))
              for c in range(NCH):
                  for (dstt, src) in ((cw0A, y_f[:, c, :]), (cw1A, y_f[:, 6 + c, :])):
                      pstt = sps.tile([4, 128], F32, tag="spt",
                                      name=f"ct_{c}_{dstt.name}")
                      nc.tensor.transpose(pstt[:], src, ident128[:])
                      nc.scalar.activation(dstt[0:4, c, :], pstt[:], AF.Copy,
                                           scale=0.5)

          # ============ PHASE 4: final combine ============
          with tc.tile_pool(name="p4", bufs=2) as p4, \
               tc.tile_pool(name="cwp", bufs=3, space="PSUM") as cwp:
              imgsel_r = p4.tile([6, NTOK], BF16, tag="imsr", bufs=1)
              nc.sync.dma_start(imgsel_r[0:4, :], imgsel_d[:])
              nc.sync.dma_start(imgsel_r[4:6, :], s01_all[:])
              col0 = 0
              for tl in P34_TILES:
                  cs = slice(col0, col0 + tl)
                  o1 = p4.tile([128, NCH, tl], BF16, tag="o1", name=f"o1_{col0}")
                  o2 = p4.tile([128, NCH, tl], BF16, tag="o2", name=f"o2_{col0}")
                  for c in range(NCH):
                      pc0 = cwp.tile([128, tl], F32, tag="cw", name=f"c0_{col0}_{c}")
                      nc.tensor.matmul(pc0[:], cw0A[:, c, :], imgsel_r[:, cs],
                                       start=True, stop=True)
                      pc1 = cwp.tile([128, tl], F32, tag="cw", name=f"c1_{col0}_{c}")
                      nc.tensor.matmul(pc1[:], cw1A[:, c, :], imgsel_r[:, cs],
                                       start=True, stop=True)
                      t0 = p4.tile([128, tl], BF16, tag="t0", name=f"t0_{col0}_{c}")
                      nc.vector.tensor_mul(t0[:], pc1[:], x2breg[:, c, cs])
                      nc.vector.tensor_add(o1[:, c, :], x1breg[:, c, cs], t0[:])
                      t1 = p4.tile([128, tl], BF16, tag="t1", name=f"t1_{col0}_{c}")
                      nc.vector.tensor_mul(t1[:], pc0[:], x1breg[:, c, cs])
                      nc.vector.tensor_add(o2[:, c, :], x2breg[:, c, cs], t1[:])
                  nc.sync.dma_start(r6(out1_d[:])[:, :, cs], o1[:])
                  nc.sync.dma_start(r6(out2_d[:])[:, :, cs], o2[:])
                  col0 += tl

    nc.compile()
    return nc


def _windowize(x):
    Bp = np.zeros((B, 70, 70, C), x.dtype)
    Bp[:, :64, :64, :] = x
    w = Bp.reshape(B, GRID, WIN, GRID, WIN, C).transpose(0, 1, 3, 2, 4, 5)
    w = w.reshape(NWIN_TOT, N, C)
    out = np.zeros((NCORE * NW, N, C), x.dtype)
    out[:NWIN_TOT] = w
    return out


def _unwindowize(perwin):
    w = perwin[:NWIN_TOT].reshape(B, GRID, GRID, WIN, WIN, C)
    w = w.transpose(0, 1, 3, 2, 4, 5).reshape(B, 70, 70, C)
    return np.ascontiguousarray(w[:, :64, :64, :])


def kernel(rgb_embedding, x_embedding, norm1_w, norm1_b, qkv_w, qkv_b,
           rel_pos_h, rel_pos_w, proj_w, proj_b, norm2_w, norm2_b,
           fc1_w, fc1_b, fc2_w, fc2_b, cw1_w, cw1_b, cw2_w, cw2_b,
           sw1_w, sw1_b, sw2_w, sw2_b):
    if "nc" not in _CACHE:
        _CACHE["nc"] = _build()
    nc = _CACHE["nc"]

    f32 = lambda a: np.ascontiguousarray(a, dtype=np.float32)
    bf16 = lambda a: np.ascontiguousarray(np.asarray(a, dtype=np.float32),
                                          ).astype(ml_dtypes.bfloat16)
    xw = _windowize(f32(x_embedding))
    rw = _windowize(f32(rgb_embedding))
    vm = np.zeros((NCORE * NW, N), np.float32)
    vh = np.minimum(np.maximum(64 - np.arange(GRID) * WIN, 0), WIN)
    wm = np.zeros((GRID, GRID, WIN, WIN), np.float32)
    for a in range(GRID):
        for b in range(GRID):
            wm[a, b, :vh[a], :vh[b]] = 1.0
    vm[:NWIN_TOT] = np.tile(wm.reshape(GRID * GRID, N), (B, 1))
    win_img = np.full(NCORE * NW, -1, np.int64)
    win_img[:NWIN_TOT] = np.arange(NWIN_TOT) // (GRID * GRID)

    n1w, n1b = f32(norm1_w), f32(norm1_b)
    n2w, n2b = f32(norm2_w), f32(norm2_b)
    qkw = f32(np.asarray(qkv_w)[:2 * C])
    vw = f32(np.asarray(qkv_w)[2 * C:])
    qkw_f = qkw * n1w[None, :]
    vw_f = vw * n1w[None, :]
    qkb = f32(qkv_b[:2 * C]) + qkw @ n1b
    qkb[:C] *= SCALE
    vb_f = f32(qkv_b[2 * C:]) + vw @ n1b
    projb_fold = f32(proj_b) + f32(proj_w) @ vb_f
    fc1w = f32(fc1_w)
    fc1w_f = fc1w * n2w[None, :]
    fc1b_f = f32(fc1_b) + fc1w @ n2b
    tab = np.concatenate([f32(rel_pos_h)[::-1], f32(rel_pos_w)[::-1]], axis=0)
    tab = np.ascontiguousarray(tab.T) * (1.0 / SCALE)
    cw1s = f32(cw1_w).T.copy()
    cw1s[:2 * C, :] *= 1.0 / (HH * WW)
    shared = dict(
        qkwT=bf16(qkw_f.T), qkb=qkb,
        vwT=bf16(vw_f.T),
        projwT=bf16(np.asarray(proj_w).T), projb=projb_fold,
        tab=tab,
        fc1wT=bf16(fc1w_f.T), fc1b=fc1b_f,
        fc2wT=bf16(np.asarray(fc2_w).T), fc2b=f32(fc2_b),
        sw1wT=bf16(np.asarray(sw1_w).T), sw1b=f32(sw1_b),
        sw2wT=bf16(np.asarray(sw2_w).T), sw2b=f32(sw2_b),
        cw2bf=f32(cw2_b),
        cwaug=bf16(np.array([[0.5], [0.0], [0.0], [0.5]], np.float32)
                   * np.ones((4, NCH * 128), np.float32)),
        vones=bf16(np.ones((98, NH), np.float32)),
    )
    cw2s = np.ascontiguousarray(f32(cw2_w).T)
    in_maps = []
    for c in range(NCORE):
        sl = slice(c * NW, (c + 1) * NW)
        xT = bf16(np.ascontiguousarray(xw[sl].reshape(NTOK, C).T))
        rT = bf16(np.ascontiguousarray(rw[sl].reshape(NTOK, C).T))
        mrow = vm[sl].reshape(1, NTOK).copy()
        imgm = np.zeros((4, NW), np.float32)
        imsel = np.zeros((4, NTOK), np.float32)
        for wloc in range(NW):
            im = win_img[c * NW + wloc]
            if im >= 0:
                imgm[im, wloc] = 1.0
                imsel[im, wloc * N:(wloc + 1) * N] = 1.0
        m = dict(shared)
        m.update(
            xT=xT, rgbT=rT, mask=bf16(mrow),
            negbig=bf16((mrow - 1.0) * 1e30),
            imgmask=imgm, imgneg=(imgm - 1.0) * 1e30, imgsel=bf16(imsel),
            cw1wTs=bf16(cw1s[:, c * 384:(c + 1) * 384]),
            cw1bs=f32(cw1_b[c * 384:(c + 1) * 384]).copy(),
            cw2rT=bf16(cw2s[c * 384:(c + 1) * 384, :]),
        )
        in_maps.append(m)

    trace = bool(os.environ.get("KERNEL_TRACE"))
    res = run_bass_kernel_spmd(nc, in_maps, list(range(NCORE)), trace=trace)
    if trace:
        _CACHE["exec_time_ns"] = res.exec_time_ns
    _CACHE["res"] = res
    o1 = np.zeros((NCORE * NW, N, C), np.float32)
    o2 = np.zeros((NCORE * NW, N, C), np.float32)
    for c in range(NCORE):
        sl = slice(c * NW, (c + 1) * NW)
        o1[sl] = res.results[c]["out1T"].astype(np.float32).T.reshape(NW, N, C)
        o2[sl] = res.results[c]["out2T"].astype(np.float32).T.reshape(NW, N, C)
    rgb_out = _unwindowize(o1)
    x_out = _unwindowize(o2)
    return rgb_out, x_out
